# revision 1
# baseline (speedup 1.0000x reference)
"""Trainium2 kernel for nn_BranchModel_9680856285960 (moe_routing).

Math: the reference scatters per-branch sparse weights into dense
(n_br, n_out, n_in) tensors, einsums against x, then takes a context-
gated masked sum over branches followed by relu.  Because the mask-
weighted branch sum commutes with the contraction over input features,
the whole model collapses to a 3-layer dense MLP

    out = relu(relu(x @ Weff1.T) @ Weff2.T) @ W3 + b3

where  Weff_l[o, i] = sum_{r,k} masks_l[ctx, r, o] * w_l[r, o, k]
                                * [idx_l[r, o, k] == i].

The effective-weight fold (a scatter-add over 5.6M index/value pairs) is
data-dependent element-granular addressing, which Trainium2 has no fast
engine for; it is done once on the host here, and the device then runs
the dense pipeline.  Weights/activations stream as fp16 (the kernel is
HBM-bound on the weight stream; fp16 halves it and runs the PE at full
rate with fp32 PSUM accumulation).

Sharding: data-parallel over batch (8 cores x 128 rows), effective
weights replicated per core, activations kept feature-major on chip.
No collectives.
"""

import os
import sys
import numpy as np

for _p in ("/opt/trn_rl_repo",):
    if os.path.isdir(_p) and _p not in sys.path:
        sys.path.append(_p)

from contextlib import ExitStack

from concourse import bass, mybir
import concourse.bacc as bacc
import concourse.tile as tile
from concourse.bass_utils import run_bass_kernel_spmd
from concourse.masks import make_identity

F32 = mybir.dt.float32
F16 = mybir.dt.float16

BATCH, NIN, NH, NOUT = 1024, 784, 2000, 10
NCORES = 8
BS = BATCH // NCORES            # 128 batch rows per core
P = 128


def _tiles(total, step):
    out, o = [], 0
    while o < total:
        out.append((o, min(step, total - o)))
        o += step
    return out


MT1 = _tiles(NIN, P)            # layer-1 contraction tiles: 6x128 + 16
MT2 = _tiles(NH, P)             # layer-2/3 contraction tiles: 15x128 + 80
NCHK = _tiles(NH, 512)          # psum output chunks: 3x512 + 464

# Exposed for the test harness: the BassKernelResults of the last run.
LAST_RESULT = None
_CACHE = {}


def _build_weff(w, idx, mask_row, n_in):
    """Fold masks + branch sum into a dense effective weight matrix.

    Weff[o, i] = sum_{r,k} mask_row[r, o] * w[r, o, k] * [idx[r, o, k] == i]
    """
    n_br, n_out, npb = w.shape
    acc = np.zeros(n_out * n_in, np.float64)
    base = (np.arange(n_out, dtype=np.int64) * n_in)[:, None]
    for r in range(n_br):
        flat = (base + idx[r].astype(np.int64)).ravel()
        vals = (w[r].astype(np.float64) * mask_row[r].astype(np.float64)[:, None]).ravel()
        acc += np.bincount(flat, weights=vals, minlength=n_out * n_in)
    return acc.reshape(n_out, n_in).astype(np.float32)


def _mlp_body(tc, xT, w1t, w2t, w3p, b3r, out):
    nc = tc.nc
    rings = [nc.sync, nc.scalar]          # the two HWDGE rings

    with ExitStack() as ctx:
        const = ctx.enter_context(tc.tile_pool(name="const", bufs=1))
        wp = ctx.enter_context(tc.tile_pool(name="wslab", bufs=1))
        act = ctx.enter_context(tc.tile_pool(name="act", bufs=1))
        pacc = ctx.enter_context(tc.tile_pool(name="pacc", bufs=1, space="PSUM"))
        ptr = ctx.enter_context(tc.tile_pool(name="ptr", bufs=1, space="PSUM"))

        ident = const.tile([P, P], F16, tag="ident")
        make_identity(nc, ident[:])

        # x host-packed as [128, n_tiles, 128] (xp[p, t, b] = xT[t*128+p, b])
        # so the load is one contiguous fast DMA, first on the sync ring --
        # it gates the first layer-1 matmul.
        xbig = const.tile([P, len(MT1), P], F16, tag="xbig")
        nc.sync.dma_start(out=xbig[:], in_=xT)
        xts = [xbig[:sz, t, :] for t, (off, sz) in enumerate(MT1)]

        b3t = const.tile([NOUT, 1], F32, tag="b3")
        nc.gpsimd.dma_start(out=b3t[:], in_=b3r)

        # W3 host-packed as [128 partitions, 16 contraction tiles, 10]
        w3t = const.tile([P, len(MT2), NOUT], F16, tag="w3")
        nc.gpsimd.dma_start(out=w3t[:], in_=w3p)

        # Stream all weight slabs up front (they all fit in SBUF); the two
        # HWDGE rings run in parallel and the PE consumes slabs as they land.
        w1s, w2s = [], []
        for t, (off, sz) in enumerate(MT1):
            slab = wp.tile([sz, NH], F16, name=f"w1s{t}", tag=f"w1s{t}")
            if t < 2:
                # column-split the first slabs so the first matmuls start on
                # the first quarter instead of waiting for the full 512KB
                for noff, nsz in NCHK:
                    rings[t % 2].dma_start(
                        out=slab[:, noff:noff + nsz],
                        in_=w1t[off:off + sz, noff:noff + nsz])
            else:
                rings[t % 2].dma_start(out=slab[:], in_=w1t[off:off + sz, :])
            w1s.append(slab[:])
        for t, (off, sz) in enumerate(MT2):
            # w2 on opposite parity from w1 so the two rings carry equal bytes
            slab = wp.tile([sz, NH], F16, name=f"w2s{t}", tag=f"w2s{t}")
            if t >= len(MT2) - 3 and sz > 64:
                # split the tail slabs so the last arrival quantum is small
                h = sz // 2
                rings[(t + 1) % 2].dma_start(out=slab[:h, :],
                                             in_=w2t[off:off + h, :])
                rings[t % 2].dma_start(out=slab[h:sz, :],
                                       in_=w2t[off + h:off + sz, :])
            else:
                rings[(t + 1) % 2].dma_start(out=slab[:],
                                             in_=w2t[off:off + sz, :])
            w2s.append(slab[:])

        # ---- Layer 1: H1 = relu(x @ Weff1.T), batch on partitions
        h1 = act.tile([P, NH], F16, tag="h1")
        ps1 = [pacc.tile([P, sz], F32, name=f"ps1_{n}", tag=f"ps{n}")
               for n, (_, sz) in enumerate(NCHK)]

        for t in range(len(MT1)):
            for n, (noff, nsz) in enumerate(NCHK):
                nc.tensor.matmul(
                    ps1[n][:],
                    lhsT=xts[t],
                    rhs=w1s[t][:, noff:noff + nsz],
                    start=(t == 0),
                    stop=(t == len(MT1) - 1),
                )
        for n, (noff, nsz) in enumerate(NCHK):
            nc.vector.tensor_scalar_max(h1[:, noff:noff + nsz], ps1[n][:], 0.0)

        # Transpose H1 to feature-major tiles for the layer-2 contraction
        pts = [ptr.tile([P, P], F16, name=f"pt{i}", tag=f"pt{i}")
               for i in range(3)]
        h1Tb = act.tile([P, len(MT2), P], F16, tag="h1Tb")
        h1T = []
        for j, (off, sz) in enumerate(MT2):
            pt = pts[j % 3]
            nc.tensor.transpose(pt[:sz, :], h1[:, off:off + sz], ident[:])
            nc.vector.tensor_copy(h1Tb[:sz, j, :], pt[:sz, :])
            h1T.append(h1Tb[:sz, j, :])

        # ---- Layer 2: H2 = relu(H1 @ Weff2.T)
        h2 = act.tile([P, NH], F16, tag="h2")
        ps2 = [pacc.tile([P, sz], F32, name=f"ps2_{n}", tag=f"ps{n}")
               for n, (_, sz) in enumerate(NCHK)]
        for t in range(len(MT2)):
            for n, (noff, nsz) in enumerate(NCHK):
                nc.tensor.matmul(
                    ps2[n][:],
                    lhsT=h1T[t],
                    rhs=w2s[t][:, noff:noff + nsz],
                    start=(t == 0),
                    stop=(t == len(MT2) - 1),
                )
        # Per-j-tile relu (alternating DVE/ACT) so each transpose can start
        # as soon as its 128 columns are ready — this chain is the kernel tail.
        for j, (off, sz) in enumerate(MT2):
            n = j // 4
            csl = slice(off - NCHK[n][0], off - NCHK[n][0] + sz)
            if j % 2 == 0:
                nc.vector.tensor_scalar_max(h2[:, off:off + sz],
                                            ps2[n][:, csl], 0.0)
            else:
                nc.scalar.activation(h2[:, off:off + sz], ps2[n][:, csl],
                                     mybir.ActivationFunctionType.Relu)

        # Transpose H2 for the layer-3 contraction (copies split DVE/ACT to
        # shorten the end-of-kernel critical path)
        h2Tb = act.tile([P, len(MT2), P], F16, tag="h2Tb")
        h2T = []
        for j, (off, sz) in enumerate(MT2):
            pt = pts[j % 3]
            nc.tensor.transpose(pt[:sz, :], h2[:, off:off + sz], ident[:])
            if j % 4 == 3:
                nc.scalar.copy(h2Tb[:sz, j, :], pt[:sz, :])
            else:
                nc.vector.tensor_copy(h2Tb[:sz, j, :], pt[:sz, :])
            h2T.append(h2Tb[:sz, j, :])

        # ---- Layer 3: outT = W3.T @ H2.T + b3.  Transposed orientation:
        # w3 is the (tiny) stationary operand and the output lands as
        # [10, 128], so the final DRAM write is 10 x 512B descriptors
        # instead of 128 x 40B (the host un-transposes, pure layout).
        ps3 = pacc.tile([NOUT, P], F32, tag="ps3")
        for t, (off, sz) in enumerate(MT2):
            nc.tensor.matmul(
                ps3[:],
                lhsT=w3t[:sz, t, :],
                rhs=h2T[t],
                start=(t == 0),
                stop=(t == len(MT2) - 1),
            )
        o = act.tile([NOUT, P], F32, tag="o")
        nc.vector.tensor_add(o[:], ps3[:], b3t[:].to_broadcast([NOUT, P]))
        nc.sync.dma_start(out=out, in_=o[:])


def _get_program():
    if "nc" in _CACHE:
        return _CACHE["nc"]
    nc = bacc.Bacc("TRN2", target_bir_lowering=False, debug=False,
                   enable_asserts=False, enable_partition_id=False,
                   num_devices=NCORES)
    xT = nc.dram_tensor("xT", [P, len(MT1), BS], F16,
                        kind="ExternalInput").ap()
    w1t = nc.dram_tensor("w1t", [NIN, NH], F16, kind="ExternalInput").ap()
    w2t = nc.dram_tensor("w2t", [NH, NH], F16, kind="ExternalInput").ap()
    w3p = nc.dram_tensor("w3p", [P, len(MT2), NOUT], F16,
                         kind="ExternalInput").ap()
    b3r = nc.dram_tensor("b3r", [NOUT, 1], F32, kind="ExternalInput").ap()
    out = nc.dram_tensor("out", [NOUT, BS], F32, kind="ExternalOutput").ap()
    with tile.TileContext(nc) as tc:
        _mlp_body(tc, xT, w1t, w2t, w3p, b3r, out)
    nc.compile()
    _CACHE["nc"] = nc
    return nc


def kernel(x, w1, idx1, w2, idx2, masks1, masks2, W3, b3, context):
    global LAST_RESULT
    x = np.ascontiguousarray(np.asarray(x, dtype=np.float32))
    ctxi = int(np.asarray(context))

    weff1 = _build_weff(np.asarray(w1), np.asarray(idx1),
                        np.asarray(masks1)[ctxi], NIN)
    weff2 = _build_weff(np.asarray(w2), np.asarray(idx2),
                        np.asarray(masks2)[ctxi], NH)
    w1t = np.ascontiguousarray(weff1.T.astype(np.float16))    # (784, 2000)
    w2t = np.ascontiguousarray(weff2.T.astype(np.float16))    # (2000, 2000)

    # W3 packed to [128, n_tiles, 10]: w3p[m, t, :] = W3[t*128 + m, :]
    w3f = np.asarray(W3).astype(np.float16)
    w3p = np.zeros((P, len(MT2), NOUT), np.float16)
    for t, (off, sz) in enumerate(MT2):
        w3p[:sz, t, :] = w3f[off:off + sz, :]
    b3r = np.ascontiguousarray(
        np.asarray(b3, dtype=np.float32).reshape(NOUT, 1))

    try:
        import antenv.axon_hooks  # noqa: F401
    except Exception:
        os.environ.setdefault("BASS_NEVER_TRACE", "1")

    nc = _get_program()
    in_maps = []
    for c in range(NCORES):
        xs = x[c * BS:(c + 1) * BS].T.astype(np.float16)   # (784, 128)
        xT = np.zeros((P, len(MT1), BS), np.float16)
        for t, (off, sz) in enumerate(MT1):
            xT[:sz, t, :] = xs[off:off + sz, :]
        in_maps.append({"xT": xT, "w1t": w1t, "w2t": w2t, "w3p": w3p,
                        "b3r": b3r})

    LAST_RESULT = run_bass_kernel_spmd(nc, in_maps, list(range(NCORES)))
    return np.concatenate(
        [LAST_RESULT.results[c]["out"].T for c in range(NCORES)], axis=0)



# revision 5
# speedup vs baseline: 1.1417x; 1.1417x over previous
"""Trainium2 kernel for nn_BranchModel_9680856285960 (moe_routing).

Math: the reference scatters per-branch sparse weights into dense
(n_br, n_out, n_in) tensors, einsums against x, then takes a context-
gated masked sum over branches followed by relu.  Because the mask-
weighted branch sum commutes with the contraction over input features,
the whole model collapses to a 3-layer dense MLP

    out = relu(relu(x @ Weff1.T) @ Weff2.T) @ W3 + b3

where  Weff_l[o, i] = sum_{r,k} masks_l[ctx, r, o] * w_l[r, o, k]
                                * [idx_l[r, o, k] == i].

The effective-weight fold (a scatter-add over 5.6M index/value pairs) is
data-dependent element-granular addressing, which Trainium2 has no fast
engine for; it is done once on the host, and the device runs the dense
pipeline.

Two exact reductions beyond the baseline:
  * Dead-unit pruning: with 80% gate sparsity, ~10.7% of hidden units
    have ALL branches masked -> their Weff row is identically zero and
    the unit contributes nothing.  Those rows/columns are dropped
    exactly (h = relu(0) = 0), shrinking both layers' weights ~19%.
  * Weights are host-packed partition-major, grouped by output-column
    chunk, so each chunk is ONE ~1-1.6MB DMA with >=4KB-per-partition
    descriptors (HBM line rate), and compute consumes chunks as they
    land instead of waiting on dozens of small semaphore-chained DMAs.

Sharding: data-parallel over batch (8 cores x 128 rows), effective
weights replicated per core, fp16 on the wire, fp32 PSUM accumulation.
No collectives.
"""

import os
import sys
import numpy as np

for _p in ("/opt/trn_rl_repo",):
    if os.path.isdir(_p) and _p not in sys.path:
        sys.path.append(_p)

from contextlib import ExitStack

from concourse import bass, mybir
import concourse.bacc as bacc
import concourse.tile as tile
from concourse.bass_utils import run_bass_kernel_spmd
from concourse.masks import make_identity

F32 = mybir.dt.float32
F16 = mybir.dt.float16

BATCH, NIN, NH, NOUT = 1024, 784, 2000, 10
NCORES = 8
BS = BATCH // NCORES            # 128 batch rows per core
P = 128
KT1 = 7                         # L1 contraction tiles (784 -> 7x128 padded)

# Exposed for the test harness: the BassKernelResults of the last run.
LAST_RESULT = None
_CACHE = {}


def _chunks(n_cols):
    """Output-column chunks: small first chunk (starts PE early), the
    rest 512 wide (one PSUM bank), all multiples of 128."""
    assert n_cols % 128 == 0
    out = []
    first = min(256, n_cols)
    out.append((0, first))
    off = first
    while off < n_cols:
        w = min(512, n_cols - off)
        out.append((off, w))
        off += w
    return out


def _build_weff(w, idx, mask_row, n_in):
    """Fold masks + branch sum into a dense effective weight matrix.

    Weff[o, i] = sum_{r,k} mask_row[r, o] * w[r, o, k] * [idx[r, o, k] == i]
    """
    n_br, n_out, npb = w.shape
    acc = np.zeros(n_out * n_in, np.float64)
    base = (np.arange(n_out, dtype=np.int64) * n_in)[:, None]
    for r in range(n_br):
        flat = (base + idx[r].astype(np.int64)).ravel()
        vals = (w[r].astype(np.float64) * mask_row[r].astype(np.float64)[:, None]).ravel()
        acc += np.bincount(flat, weights=vals, minlength=n_out * n_in)
    return acc.reshape(n_out, n_in).astype(np.float32)


def _pack_chunks(wt, kt, chunks):
    """Pack wt (n_in_padded=kt*128 rows, n_cols) into the on-wire layout:
    flat[p, chunk-major: (c, t, col)] = wt[t*128+p, c0+col], so one chunk
    is per-partition contiguous (kt * w * 2 bytes)."""
    n_cols = wt.shape[1]
    total = kt * n_cols
    out = np.zeros((P, total), np.float16)
    pos = 0
    for (c0, w) in chunks:
        blk = wt[:, c0:c0 + w].reshape(kt, P, w)      # [t, p, col]
        out[:, pos:pos + kt * w] = blk.transpose(1, 0, 2).reshape(P, kt * w)
        pos += kt * w
    return out


def _mlp_body(tc, n1t, n2t, xT, w1pk, w2pk, w3p, b3r, out):
    nc = tc.nc
    n1, n2 = n1t * P, n2t * P
    ch1 = _chunks(n1)
    ch2 = _chunks(n2)
    rings = [nc.sync, nc.scalar]

    with ExitStack() as ctx:
        const = ctx.enter_context(tc.tile_pool(name="const", bufs=1))
        wp = ctx.enter_context(tc.tile_pool(name="wslab", bufs=1))
        act = ctx.enter_context(tc.tile_pool(name="act", bufs=1))
        pacc = ctx.enter_context(tc.tile_pool(name="pacc", bufs=1, space="PSUM"))
        ptr = ctx.enter_context(tc.tile_pool(name="ptr", bufs=1, space="PSUM"))

        ident = const.tile([P, P], F16, tag="ident")
        make_identity(nc, ident[:])

        # x first (stationary operand of every L1 matmul): one small DMA.
        xbig = const.tile([P, KT1, BS], F16, tag="xbig")
        nc.sync.dma_start(out=xbig[:], in_=xT)
        xts = [xbig[:, t, :] for t in range(KT1)]

        b3t = const.tile([NOUT, 1], F32, tag="b3")
        nc.gpsimd.dma_start(out=b3t[:], in_=b3r)
        w3t = const.tile([P, n2t, NOUT], F16, tag="w3")
        nc.gpsimd.dma_start(out=w3t[:], in_=w3p)

        # Weight chunk slabs: each chunk is ONE large per-partition-
        # contiguous DMA.  Alternate rings so both carry ~half the bytes,
        # in consumption order.
        w1s, pos = [], 0
        for i, (c0, w) in enumerate(ch1):
            slab = wp.tile([P, KT1, w], F16, name=f"w1s{i}", tag=f"w1s{i}")
            rings[i % 2].dma_start(out=slab[:], in_=w1pk[:, pos:pos + KT1 * w])
            w1s.append(slab)
            pos += KT1 * w
        w2s, pos = [], 0
        for i, (c0, w) in enumerate(ch2):
            slab = wp.tile([P, n1t, w], F16, name=f"w2s{i}", tag=f"w2s{i}")
            rings[(i + 1) % 2].dma_start(out=slab[:], in_=w2pk[:, pos:pos + n1t * w])
            w2s.append(slab)
            pos += n1t * w

        # ---- Layer 1: H1 = relu(x @ Weff1.T), batch on partitions,
        # chunk-major so compute follows the w1 stream.
        h1 = act.tile([P, n1], F16, tag="h1")
        h1Tb = act.tile([P, n1t, P], F16, tag="h1Tb")
        pts = [ptr.tile([P, P], F16, name=f"pt{i}", tag=f"pt{i}") for i in range(3)]
        pti = 0
        for i, (c0, w) in enumerate(ch1):
            ps = pacc.tile([P, w], F32, name=f"ps1_{i}", tag=f"ps{i % 2}")
            for t in range(KT1):
                nc.tensor.matmul(ps[:], lhsT=xts[t], rhs=w1s[i][:, t, :],
                                 start=(t == 0), stop=(t == KT1 - 1))
            if i % 2 == 0:
                nc.vector.tensor_scalar_max(h1[:, c0:c0 + w], ps[:], 0.0)
            else:
                nc.scalar.activation(h1[:, c0:c0 + w], ps[:],
                                     mybir.ActivationFunctionType.Relu)
            for j in range(w // P):
                jg = (c0 // P) + j
                pt = pts[pti % 3]; pti += 1
                nc.tensor.transpose(pt[:], h1[:, jg * P:(jg + 1) * P], ident[:])
                if pti % 2 == 0:
                    nc.scalar.copy(h1Tb[:, jg, :], pt[:])
                else:
                    nc.vector.tensor_copy(h1Tb[:, jg, :], pt[:])

        # ---- Layer 2: H2 = relu(H1 @ Weff2.T), chunk-major; L3 partial
        # accumulation runs per chunk so the kernel tail stays short.
        h2s = [act.tile([P, P], F16, name=f"h2_{i}", tag=f"h2_{i}")
               for i in range(3)]  # rotating per-j-tile staging
        h2Tb = act.tile([P, n2t, P], F16, tag="h2Tb")
        ps3 = ptr.tile([NOUT, P], F32, tag="ps3")
        for i, (c0, w) in enumerate(ch2):
            ps = pacc.tile([P, w], F32, name=f"ps2_{i}", tag=f"ps{i % 2}")
            for t in range(n1t):
                nc.tensor.matmul(ps[:], lhsT=h1Tb[:, t, :], rhs=w2s[i][:, t, :],
                                 start=(t == 0), stop=(t == n1t - 1))
            for j in range(w // P):
                jg = (c0 // P) + j
                pt = pts[pti % 3]
                h2 = h2s[pti % 3]
                pti += 1
                # relu straight out of PSUM, per 128-col group
                if pti % 2 == 0:
                    nc.vector.tensor_scalar_max(h2[:], ps[:, j * P:(j + 1) * P], 0.0)
                else:
                    nc.scalar.activation(h2[:], ps[:, j * P:(j + 1) * P],
                                         mybir.ActivationFunctionType.Relu)
                nc.tensor.transpose(pt[:], h2[:], ident[:])
                if pti % 2 == 0:
                    nc.scalar.copy(h2Tb[:, jg, :], pt[:])
                else:
                    nc.vector.tensor_copy(h2Tb[:, jg, :], pt[:])
                # ---- Layer 3 partial: accumulate W3.T @ H2.T tile
                nc.tensor.matmul(ps3[:], lhsT=w3t[:, jg, :], rhs=h2Tb[:, jg, :],
                                 start=(jg == 0), stop=(jg == n2t - 1))

        o = act.tile([NOUT, P], F32, tag="o")
        nc.vector.tensor_add(o[:], ps3[:], b3t[:].to_broadcast([NOUT, P]))
        nc.sync.dma_start(out=out, in_=o[:])


def _get_program(n1t, n2t):
    key = (n1t, n2t)
    if key in _CACHE:
        return _CACHE[key]
    nc = bacc.Bacc("TRN2", target_bir_lowering=False, debug=False,
                   enable_asserts=False, enable_partition_id=False,
                   num_devices=NCORES)
    n1, n2 = n1t * P, n2t * P
    xT = nc.dram_tensor("xT", [P, KT1, BS], F16, kind="ExternalInput").ap()
    w1pk = nc.dram_tensor("w1pk", [P, KT1 * n1], F16, kind="ExternalInput").ap()
    w2pk = nc.dram_tensor("w2pk", [P, n1t * n2], F16, kind="ExternalInput").ap()
    w3p = nc.dram_tensor("w3p", [P, n2t, NOUT], F16, kind="ExternalInput").ap()
    b3r = nc.dram_tensor("b3r", [NOUT, 1], F32, kind="ExternalInput").ap()
    out = nc.dram_tensor("out", [NOUT, BS], F32, kind="ExternalOutput").ap()
    with tile.TileContext(nc) as tc:
        _mlp_body(tc, n1t, n2t, xT, w1pk, w2pk, w3p, b3r, out)
    nc.compile()
    _CACHE[key] = nc
    return nc


def kernel(x, w1, idx1, w2, idx2, masks1, masks2, W3, b3, context):
    global LAST_RESULT
    x = np.ascontiguousarray(np.asarray(x, dtype=np.float32))
    ctxi = int(np.asarray(context))

    weff1 = _build_weff(np.asarray(w1), np.asarray(idx1),
                        np.asarray(masks1)[ctxi], NIN)
    weff2 = _build_weff(np.asarray(w2), np.asarray(idx2),
                        np.asarray(masks2)[ctxi], NH)

    # Exact dead-unit pruning: units whose Weff row is identically zero
    # output relu(0)=0 and contribute nothing downstream.
    a1 = np.flatnonzero(np.abs(weff1).sum(1))
    a2 = np.flatnonzero(np.abs(weff2).sum(1))
    n1t = max(1, -(-len(a1) // P))
    n2t = max(1, -(-len(a2) // P))
    n1, n2 = n1t * P, n2t * P

    w1p = np.zeros((n1, NIN), np.float32); w1p[:len(a1)] = weff1[a1]
    w2p = np.zeros((n2, n1), np.float32)
    w2p[:len(a2), :len(a1)] = weff2[np.ix_(a2, a1)]
    W3p = np.zeros((n2, NOUT), np.float32); W3p[:len(a2)] = np.asarray(W3)[a2]

    # w1 transposed to (n_in, n1), rows zero-padded to 7*128
    w1t = np.zeros((KT1 * P, n1), np.float32); w1t[:NIN] = w1p.T
    w2t = w2p.T                                            # (n1, n2)

    w1pk = _pack_chunks(w1t.astype(np.float16), KT1, _chunks(n1))
    w2pk = _pack_chunks(w2t.astype(np.float16), n1t, _chunks(n2))

    w3p = np.zeros((P, n2t, NOUT), np.float16)
    w3p[:, :, :] = W3p.astype(np.float16).reshape(n2t, P, NOUT).transpose(1, 0, 2)
    b3r = np.ascontiguousarray(
        np.asarray(b3, dtype=np.float32).reshape(NOUT, 1))

    try:
        import antenv.axon_hooks  # noqa: F401
    except Exception:
        os.environ.setdefault("BASS_NEVER_TRACE", "1")

    nc = _get_program(n1t, n2t)
    in_maps = []
    for c in range(NCORES):
        xs = x[c * BS:(c + 1) * BS].T.astype(np.float16)   # (784, 128)
        xT = np.zeros((P, KT1, BS), np.float16)
        for t in range(KT1):
            rows = xs[t * P:(t + 1) * P]
            xT[:rows.shape[0], t, :] = rows
        in_maps.append({"xT": xT, "w1pk": w1pk, "w2pk": w2pk, "w3p": w3p,
                        "b3r": b3r})

    LAST_RESULT = run_bass_kernel_spmd(nc, in_maps, list(range(NCORES)))
    return np.concatenate(
        [LAST_RESULT.results[c]["out"].T for c in range(NCORES)], axis=0)
